# revision 1
# baseline (speedup 1.0000x reference)
"""Trainium2 Bass kernel (raw Bass, no Tile): per-class precision/recall sums.

Computes, for pred/gt 0-1 indicator tensors of shape [N, C]:
    intersection = sum_n pred*gt   [C]
    pred_sum     = sum_n pred      [C]
    gt_sum       = sum_n gt        [C]
    precisions   = (intersection + EPS) / (pred_sum + EPS)
    recalls      = (intersection + EPS) / (gt_sum + EPS)

Sharding: rows split across 8 NeuronCores. The host interleaves each
core's pred/gt chunks in 8-row blocks into x[R/8, 2, 8, C] so one DMA
per tile feeds both operands (each 128-element run purely pred or gt).
Each core emits a [1, 3*C] partial; the host sums partials (integer
values < 2^24, exact in fp32) and applies the epsilon math.

Device pipeline per core (memory-bound, 64 MiB HBM traffic):
  - gpsimd SWDGE DMAs cast f32 -> bf16 in flight (exact for 0/1):
    32 tiles xt[128, 4096] bf16 into 16 rotating SBUF slots.
  - TensorE does all the math:
    * ones[128,1]^T @ 512-col slices accumulate ps_sums[1,512].
    * Gram matmuls pred_run^T @ gt_run per 256-col block accumulate
      ps_gram[128,128]; diagonal entry a=(r,c) = pred.gt dot.
  - Epilogue: diag-mask ps_gram (affine_select identity), fp32
    ones-matmul column-sum -> ps_row[1,128], strided reduces fold into
    res[1,48] = [pred_sum, gt_sum, intersection].

Raw Bass because this compiler build encodes at most ONE semaphore wait
per TPB instruction: all multi-condition waits are standalone sequencer
wait_ge instructions. Correctness argument for slot recycling: the DMA
for tile t (t>=16) waits pe_sem >= t-15, i.e. PE finished reading tile
t-16 in that slot, which also implies that DMA t-16 completed.
Per-slot DMA-completion sems make PE's data waits exact even if the
runtime spreads DMAs across queues that complete out of order.
"""

from contextlib import ExitStack

import numpy as np

N_CORES = 8
N_ROWS, C = 4194304, 16
ROWS_PER_CORE = N_ROWS // N_CORES  # 524288
EPS = np.float32(1e-6)

P = 128
ELEMS_PER_CORE = ROWS_PER_CORE * 2 * C      # 16777216
FREE = 8192          # bf16 elements per partition per tile
TILE_ELEMS = P * FREE                       # 1048576
N_TILES = ELEMS_PER_CORE // TILE_ELEMS      # 16
N_SLOTS = 10
MM_FREE = 512
N_SUM_SLICES = FREE // MM_FREE              # 16
GRAM_BLK = 256       # (two=2, r=8, c=16)
N_GRAM_BLKS = FREE // GRAM_BLK              # 32

_CACHE = {}
LAST_RUN = None  # BassKernelResults of the most recent run (for test harness)


def _build_nc():
    import concourse.bass as bass
    import concourse.mybir as mybir

    f32 = mybir.dt.float32
    bf16 = mybir.dt.bfloat16

    nc = bass.Bass()
    x_d = nc.dram_tensor("x", [ROWS_PER_CORE // 8, 2, 8, C], f32,
                         kind="ExternalInput")
    out_d = nc.dram_tensor("out", [1, 3 * C], f32, kind="ExternalOutput")

    x_t = x_d[:, :, :, :].rearrange("(t p f) two r c -> t p (f two r c)",
                                    p=P, f=FREE // GRAM_BLK)

    ctx = ExitStack()
    with ctx:
        ones_b = ctx.enter_context(nc.sbuf_tensor("ones_b", [P, 1], bf16))
        ones_f = ctx.enter_context(nc.sbuf_tensor("ones_f", [P, 1], f32))
        onesI = ctx.enter_context(nc.sbuf_tensor("onesI", [P, P], f32))
        ident = ctx.enter_context(nc.sbuf_tensor("ident", [P, P], f32))
        diag = ctx.enter_context(nc.sbuf_tensor("diag", [P, P], f32))
        sum4 = ctx.enter_context(nc.sbuf_tensor("sum4", [1, 4 * C], f32))
        res = ctx.enter_context(nc.sbuf_tensor("res", [1, 3 * C], f32))
        slots = [
            ctx.enter_context(nc.sbuf_tensor(f"xt{s}", [P, FREE], bf16))
            for s in range(N_SLOTS)
        ]

        ps_sums = ctx.enter_context(nc.psum_tensor([1, MM_FREE], f32))
        ps_gram = ctx.enter_context(nc.psum_tensor([P, P], f32))
        ps_row = ctx.enter_context(nc.psum_tensor([1, P], f32))

        slot_sems = [
            ctx.enter_context(nc.semaphore(name=f"slot{s}"))
            for s in range(N_SLOTS)
        ]
        qsems = [
            ctx.enter_context(nc.semaphore(name=f"q{k}"))
            for k in range(4)
        ]
        pe_sem = ctx.enter_context(nc.semaphore(name="pe"))
        dve_sem = ctx.enter_context(nc.semaphore(name="dve"))
        pool_sem = ctx.enter_context(nc.semaphore(name="pool"))
        out_sem = ctx.enter_context(nc.semaphore(name="outd"))
        block = ctx.enter_context(nc.Block())

        @block.gpsimd
        def _(gpsimd):
            gpsimd.memset(onesI[:], 1.0)
            gpsimd.affine_select(ident[:], onesI[:], [[1, P]],
                                 mybir.AluOpType.is_equal, 0.0,
                                 base=0, channel_multiplier=-1)
            gpsimd.nop().then_inc(pool_sem, 1)
            for t in range(N_TILES):
                s = t % N_SLOTS
                if t >= N_SLOTS:
                    # PE finished reading the previous occupant of this slot
                    gpsimd.wait_ge(pe_sem, t - N_SLOTS + 1)
                if t < N_TILES - 1:
                    gpsimd.dma_start(slots[s][:], x_t[t]).then_inc(
                        slot_sems[s], 16)
                else:
                    # last tile: 4 quarter-DMAs so PE can chase the stream
                    # and finish right after the final byte lands
                    q = FREE // 4
                    for k in range(4):
                        gpsimd.dma_start(
                            slots[s][:, k * q:(k + 1) * q],
                            x_t[t][:, k * q:(k + 1) * q],
                        ).then_inc(qsems[k], 16)
            # final output DMA after DVE finishes the epilogue
            gpsimd.wait_ge(dve_sem, 3)
            gpsimd.dma_start(out_d[:, :], res[:]).then_inc(out_sem, 16)
            gpsimd.wait_ge(out_sem, 16)

        @block.vector
        def _(vector):
            vector.memset(ones_b[:], 1.0)
            vector.memset(ones_f[:], 1.0)
            vector.nop().then_inc(dve_sem, 1)
            # epilogue part 1: after all accumulation matmuls
            vector.wait_ge(pe_sem, N_TILES)
            vector.wait_ge(pool_sem, 1)
            vector.tensor_mul(diag[:], ps_gram[:, :], ident[:])
            vector.tensor_reduce(
                sum4[:],
                ps_sums[:, :].rearrange("p (b2 two r c) -> p b2 two c r",
                                        b2=2, two=2, r=8, c=C),
                axis=mybir.AxisListType.X, op=mybir.AluOpType.add)
            vector.tensor_reduce(
                res[0:1, 0:2 * C],
                sum4[:, :].rearrange("p (b2 tc) -> p tc b2", b2=2, tc=2 * C),
                axis=mybir.AxisListType.X, op=mybir.AluOpType.add)
            vector.nop().then_inc(dve_sem, 1)  # diag + sums folded
            # epilogue part 2: after PE's diag column-sum matmul
            vector.wait_ge(pe_sem, N_TILES + 1)
            vector.tensor_reduce(
                res[0:1, 2 * C:3 * C],
                ps_row[:, :].rearrange("p (g c) -> p c g", g=8, c=C),
                axis=mybir.AxisListType.X, op=mybir.AluOpType.add)
            vector.nop().then_inc(dve_sem, 1)

        @block.tensor
        def _(tensor):
            tensor.wait_ge(dve_sem, 1)  # ones_b / ones_f ready
            for t in range(N_TILES):
                s = t % N_SLOTS
                xt = slots[s]
                quarters = 1 if t < N_TILES - 1 else 4
                if quarters == 1:
                    tensor.wait_ge(slot_sems[s], 16 * (t // N_SLOTS + 1))
                for k in range(quarters):
                    if quarters == 4:
                        tensor.wait_ge(qsems[k], 16)
                    nsum = N_SUM_SLICES // quarters
                    ngram = N_GRAM_BLKS // quarters
                    for i in range(k * nsum, (k + 1) * nsum):
                        mm = t * N_SUM_SLICES + i
                        nc.tensor.matmul(
                            ps_sums[:, :], ones_b[:],
                            xt[:, i * MM_FREE:(i + 1) * MM_FREE],
                            start=(mm == 0),
                            stop=(mm == N_TILES * N_SUM_SLICES - 1))
                    for j in range(k * ngram, (k + 1) * ngram):
                        mm = t * N_GRAM_BLKS + j
                        base = j * GRAM_BLK
                        mminst = nc.tensor.matmul(
                            ps_gram[:, :], xt[:, base:base + P],
                            xt[:, base + P:base + 2 * P],
                            start=(mm == 0),
                            stop=(mm == N_TILES * N_GRAM_BLKS - 1))
                        if j == N_GRAM_BLKS - 1:
                            mminst.then_inc(pe_sem, 1)
            # epilogue: fp32 column-sum of masked diagonal
            tensor.wait_ge(dve_sem, 2)
            nc.tensor.matmul(ps_row[:, :], ones_f[:], diag[:],
                             start=True, stop=True).then_inc(pe_sem, 1)

    return nc


def _get_nc():
    if "nc" not in _CACHE:
        _CACHE["nc"] = _build_nc()
    return _CACHE["nc"]


def kernel(pred, gt, **run_kwargs):
    global LAST_RUN
    from concourse.bass_utils import run_bass_kernel_spmd

    pred = np.asarray(pred, dtype=np.float32)
    gt = np.asarray(gt, dtype=np.float32)
    assert pred.shape == (N_ROWS, C) and gt.shape == (N_ROWS, C)

    in_maps = []
    for i in range(N_CORES):
        sl = slice(i * ROWS_PER_CORE, (i + 1) * ROWS_PER_CORE)
        x = np.empty((ROWS_PER_CORE // 8, 2, 8, C), dtype=np.float32)
        x[:, 0, :, :] = pred[sl].reshape(-1, 8, C)
        x[:, 1, :, :] = gt[sl].reshape(-1, 8, C)
        in_maps.append({"x": x})

    nc = _get_nc()
    br = run_bass_kernel_spmd(nc, in_maps, core_ids=list(range(N_CORES)),
                              **run_kwargs)
    LAST_RUN = br

    partials = np.stack([r["out"].reshape(3 * C) for r in br.results])
    totals = partials.astype(np.float64).sum(axis=0)  # exact integers
    pred_sum = totals[0:C].astype(np.float32)
    gt_sum = totals[C:2 * C].astype(np.float32)
    intersection = totals[2 * C:3 * C].astype(np.float32)

    recalls = (intersection + EPS) / (gt_sum + EPS)
    precisions = (intersection + EPS) / (pred_sum + EPS)
    return (precisions, recalls, intersection, gt_sum, pred_sum)



# revision 11
# speedup vs baseline: 1.9793x; 1.9793x over previous
"""Trainium2 Bass kernel: per-class precision/recall sums via fp8 gram matmuls.

Computes, for pred/gt 0-1 indicator tensors of shape [N, C]:
    intersection = sum_n pred*gt   [C]
    pred_sum     = sum_n pred      [C]
    gt_sum       = sum_n gt        [C]
    precisions   = (intersection + EPS) / (pred_sum + EPS)
    recalls      = (intersection + EPS) / (gt_sum + EPS)

Sharding: rows split across 8 NeuronCores. Values are 0/1 indicators, so
the host re-encodes them losslessly as fp8_e4m3 (1 byte/elem) — 4x less
HBM traffic than the f32 originals (16.8 MiB/core, ~46 us at the per-core
~368 GB/s HBM roofline, vs 182 us for f32).

Host staging per core: x[tile=16, p=128, free=8224] fp8, where free is 32
groups of 257 cols: [pred(128) | ones(1) | gt(128)]. A group's 128 cols
are (class c, subrow r) pairs, col = c*8+r; its 128 partitions each hold
a distinct row, so one group covers 1024 rows.

Device pipeline per core:
  - 16 whole-tile DMAs (last tile in 4 quarters so compute can chase the
    stream's tail); all 16 SBUF slots are resident — no recycling.
  - TensorE: one matmul per group: lhsT = pred cols (128-wide: FWL fires),
    rhs = [ones | gt] (N=129), accumulating into ps_gram[128, 129]:
      col 0   = per-(c,r) pred sums   (weights x ones column)
      diag of cols 1..129 = per-(c,r) intersections
  - VectorE: per-tile strided reduce of the gt section -> per-(p, c)
    partial gt sums; folded across tiles at the end.
  - Epilogue: DVE masks ps_gram's diagonal (identity mask) and copies
    col 0; three tiny fp32 matmuls fold the partition axis; DVE copies
    psum to SBUF; DMA writes a [1, 272] partial vector.
  - Host: sums 8 cores' partials (exact integers in f64), folds subrows,
    applies the epsilon math.
"""

from contextlib import ExitStack

import numpy as np

N_CORES = 8
N_ROWS, C = 4194304, 16
ROWS_PER_CORE = N_ROWS // N_CORES  # 524288
EPS = np.float32(1e-6)

P = 128              # partitions; also pred/gt cols per group (16 classes x 8 subrows)
R_SUB = 8            # subrows folded into a group's column block
GCOLS = 2 * P + 1    # 257: [pred(128) | ones(1) | gt(128)]
GROUPS_PER_TILE = 32
N_TILES = ROWS_PER_CORE // (P * R_SUB * GROUPS_PER_TILE)  # 16
TILE_FREE = GROUPS_PER_TILE * GCOLS  # 8224
N_QUARTERS = 4       # last tile split so PE/DVE finish right after the stream
GROUPS_PER_QUARTER = GROUPS_PER_TILE // N_QUARTERS  # 8
N_SEGS = N_TILES - 1 + N_QUARTERS  # gt-partial segments in gtacc
OUT_COLS = 2 * P + C  # 272: [I by (c,r) | pred by (c,r) | gt by c]

_F8_ONE = np.uint8(0x38)  # 1.0 in float8_e4m3

_CACHE = {}
LAST_RUN = None  # BassKernelResults of the most recent run (for test harness)


def _build_nc(n_tiles=N_TILES, groups_per_tile=GROUPS_PER_TILE):
    import concourse.bass as bass
    import concourse.mybir as mybir

    f32 = mybir.dt.float32
    fp8 = mybir.dt.float8e4

    tile_free = groups_per_tile * GCOLS
    groups_per_quarter = groups_per_tile // N_QUARTERS
    n_segs = n_tiles - 1 + N_QUARTERS
    n_groups = n_tiles * groups_per_tile

    nc = bass.Bass()
    x_d = nc.dram_tensor("x", [n_tiles, P, tile_free], fp8, kind="ExternalInput")
    out_d = nc.dram_tensor("out", [1, OUT_COLS], f32, kind="ExternalOutput")

    ctx = ExitStack()
    with ctx:
        ones_f = ctx.enter_context(nc.sbuf_tensor("ones_f", [P, 1], f32))
        onesI = ctx.enter_context(nc.sbuf_tensor("onesI", [P, P], f32))
        ident = ctx.enter_context(nc.sbuf_tensor("ident", [P, P], f32))
        diagm = ctx.enter_context(nc.sbuf_tensor("diagm", [P, P + 1], f32))
        gtacc = ctx.enter_context(nc.sbuf_tensor("gtacc", [P, n_segs * C], f32))
        gt16 = ctx.enter_context(nc.sbuf_tensor("gt16", [P, C], f32))
        res = ctx.enter_context(nc.sbuf_tensor("res", [1, OUT_COLS], f32))
        slots = [
            ctx.enter_context(nc.sbuf_tensor(f"xt{t}", [P, tile_free], fp8))
            for t in range(n_tiles)
        ]

        ps_gram = ctx.enter_context(nc.psum_tensor([P, P + 1], f32))
        ps_epiA = ctx.enter_context(nc.psum_tensor([1, P], f32))
        ps_epiB = ctx.enter_context(nc.psum_tensor([1, P], f32))
        ps_epiC = ctx.enter_context(nc.psum_tensor([1, C], f32))

        tsems = [
            ctx.enter_context(nc.semaphore(name=f"t{t}"))
            for t in range(n_tiles - 1)
        ]
        qsems = [
            ctx.enter_context(nc.semaphore(name=f"q{k}"))
            for k in range(N_QUARTERS)
        ]
        pe_sem = ctx.enter_context(nc.semaphore(name="pe"))
        dve_sem = ctx.enter_context(nc.semaphore(name="dve"))
        pool_sem = ctx.enter_context(nc.semaphore(name="pool"))
        v_sem = ctx.enter_context(nc.semaphore(name="vself"))
        out_sem = ctx.enter_context(nc.semaphore(name="outd"))
        block = ctx.enter_context(nc.Block())

        def gt_view(slot, f0, f1):
            # [p, c, f, r] view of the gt sections of groups [f0, f1)
            v = slot[:, :].rearrange("p (f col) -> p f col", f=groups_per_tile)
            v = v[:, f0:f1, P + 1:GCOLS]
            return v.rearrange("p f (c r) -> p c f r", r=R_SUB)

        @block.gpsimd
        def _(gpsimd):
            for t in range(n_tiles - 1):
                gpsimd.dma_start(slots[t][:], x_d[t]).then_inc(tsems[t], 16)
            last = n_tiles - 1
            qf = tile_free // N_QUARTERS
            for k in range(N_QUARTERS):
                gpsimd.dma_start(
                    slots[last][:, k * qf:(k + 1) * qf],
                    x_d[last][:, k * qf:(k + 1) * qf],
                ).then_inc(qsems[k], 16)
            gpsimd.memset(onesI[:], 1.0).then_inc(pool_sem, 1)
            gpsimd.wait_ge(pool_sem, 1)  # Q7 cores race back-to-back ops
            gpsimd.affine_select(ident[:], onesI[:], [[1, P]],
                                 mybir.AluOpType.is_equal, 0.0,
                                 base=0,
                                 channel_multiplier=-1).then_inc(pool_sem, 1)
            # final output DMA after DVE copies psum -> res
            gpsimd.wait_ge(dve_sem, 6)
            gpsimd.dma_start(out_d[:, :], res[:]).then_inc(out_sem, 16)
            gpsimd.wait_ge(out_sem, 16)

        @block.vector
        def _(vector):
            vector.memset(ones_f[:], 1.0)
            for t in range(n_tiles - 1):
                vector.wait_ge(tsems[t], 16)
                vector.tensor_reduce(
                    gtacc[:, t * C:(t + 1) * C],
                    gt_view(slots[t], 0, groups_per_tile),
                    axis=mybir.AxisListType.XY,
                    op=mybir.AluOpType.add).then_inc(v_sem, 1)
            for k in range(N_QUARTERS):
                vector.wait_ge(qsems[k], 16)
                seg = n_tiles - 1 + k
                vector.tensor_reduce(
                    gtacc[:, seg * C:(seg + 1) * C],
                    gt_view(slots[n_tiles - 1], k * groups_per_quarter,
                            (k + 1) * groups_per_quarter),
                    axis=mybir.AxisListType.XY,
                    op=mybir.AluOpType.add).then_inc(v_sem, 1)
            # self-sync: gtacc writes must land before the fold reads them
            vector.wait_ge(v_sem, n_segs)
            vector.tensor_reduce(
                gt16[:, :],
                gtacc[:, :].rearrange("p (s c) -> p c s", c=C),
                axis=mybir.AxisListType.X,
                op=mybir.AluOpType.add).then_inc(dve_sem, 1)
            # mask ps_gram's diagonal; copy the pred-sums column
            vector.wait_ge(pool_sem, 2)
            vector.wait_ge(pe_sem, 1)
            vector.tensor_mul(diagm[:, 1:P + 1], ps_gram[:, 1:P + 1],
                              ident[:]).then_inc(dve_sem, 1)
            vector.tensor_scalar_mul(diagm[:, 0:1], ps_gram[:, 0:1],
                                     1.0).then_inc(dve_sem, 1)
            # copy epilogue psums -> res after PE's fold matmuls
            vector.wait_ge(pe_sem, 2)
            vector.tensor_scalar_mul(res[:, 0:P], ps_epiA[:, :],
                                     1.0).then_inc(dve_sem, 1)
            vector.tensor_scalar_mul(res[:, P:2 * P], ps_epiB[:, :],
                                     1.0).then_inc(dve_sem, 1)
            vector.tensor_scalar_mul(res[:, 2 * P:OUT_COLS], ps_epiC[:, :],
                                     1.0).then_inc(dve_sem, 1)

        @block.tensor
        def _(tensor):
            mm = 0

            def issue(t, g):
                nonlocal mm
                base = g * GCOLS
                inst = nc.tensor.matmul(
                    ps_gram[:, :],
                    slots[t][:, base:base + P],
                    slots[t][:, base + P:base + GCOLS],
                    start=(mm == 0), stop=(mm == n_groups - 1))
                if mm == n_groups - 1:
                    inst.then_inc(pe_sem, 1)
                mm += 1

            for t in range(n_tiles - 1):
                tensor.wait_ge(tsems[t], 16)
                for g in range(groups_per_tile):
                    issue(t, g)
            for k in range(N_QUARTERS):
                tensor.wait_ge(qsems[k], 16)
                for g in range(k * groups_per_quarter,
                               (k + 1) * groups_per_quarter):
                    issue(n_tiles - 1, g)
            # epilogue: fold the partition axis of the three partial sets
            tensor.wait_ge(pool_sem, 2)
            tensor.wait_ge(dve_sem, 3)
            nc.tensor.matmul(ps_epiA[:, :], ones_f[:], diagm[:, 1:P + 1],
                             start=True, stop=True)
            nc.tensor.matmul(ps_epiB[:, :], diagm[:, 0:1], ident[:],
                             start=True, stop=True)
            nc.tensor.matmul(ps_epiC[:, :], ones_f[:], gt16[:, :],
                             start=True, stop=True).then_inc(pe_sem, 1)

    return nc


def _pack_core(pred_c, gt_c, n_tiles=N_TILES, groups_per_tile=GROUPS_PER_TILE):
    """Stage one core's rows as [n_tiles, P, tile_free] fp8 bytes (uint8)."""
    shp = (n_tiles, P, groups_per_tile, R_SUB, C)
    pc = np.asarray(pred_c).reshape(shp)
    gc = np.asarray(gt_c).reshape(shp)
    X = np.empty((n_tiles, P, groups_per_tile, GCOLS), np.uint8)
    # cols are (c, r) pairs, col = c*R_SUB + r -> transpose r and c
    X[..., 0:P] = (pc.transpose(0, 1, 2, 4, 3) != 0).reshape(
        n_tiles, P, groups_per_tile, P) * _F8_ONE
    X[..., P] = _F8_ONE
    X[..., P + 1:GCOLS] = (gc.transpose(0, 1, 2, 4, 3) != 0).reshape(
        n_tiles, P, groups_per_tile, P) * _F8_ONE
    return X.reshape(n_tiles, P, groups_per_tile * GCOLS)


def _unpack_out(partials):
    """partials: [OUT_COLS] f64 summed over cores -> (I, pred_sum, gt_sum)."""
    inter = partials[0:P].reshape(C, R_SUB).sum(axis=1)
    pred_sum = partials[P:2 * P].reshape(C, R_SUB).sum(axis=1)
    gt_sum = partials[2 * P:2 * P + C]
    return inter, pred_sum, gt_sum


def _get_nc():
    if "nc" not in _CACHE:
        _CACHE["nc"] = _build_nc()
    return _CACHE["nc"]


def kernel(pred, gt, **run_kwargs):
    global LAST_RUN
    import ml_dtypes
    from concourse.bass_utils import run_bass_kernel_spmd

    pred = np.asarray(pred)
    gt = np.asarray(gt)
    assert pred.shape == (N_ROWS, C) and gt.shape == (N_ROWS, C)

    in_maps = []
    for i in range(N_CORES):
        sl = slice(i * ROWS_PER_CORE, (i + 1) * ROWS_PER_CORE)
        X = _pack_core(pred[sl], gt[sl])
        in_maps.append({"x": X.view(ml_dtypes.float8_e4m3)})

    nc = _get_nc()
    br = run_bass_kernel_spmd(nc, in_maps, core_ids=list(range(N_CORES)),
                              **run_kwargs)
    LAST_RUN = br

    partials = np.stack([r["out"].reshape(OUT_COLS) for r in br.results])
    totals = partials.astype(np.float64).sum(axis=0)  # exact integers
    inter, pred_sum, gt_sum = _unpack_out(totals)
    inter = inter.astype(np.float32)
    pred_sum = pred_sum.astype(np.float32)
    gt_sum = gt_sum.astype(np.float32)

    recalls = (inter + EPS) / (gt_sum + EPS)
    precisions = (inter + EPS) / (pred_sum + EPS)
    return (precisions, recalls, inter, gt_sum, pred_sum)


# revision 13
# speedup vs baseline: 2.6645x; 1.3462x over previous
"""Trainium2 Bass kernel: per-class precision/recall sums via fp8 gram matmuls.

Computes, for pred/gt 0-1 indicator tensors of shape [N, C]:
    intersection = sum_n pred*gt   [C]
    pred_sum     = sum_n pred      [C]
    gt_sum       = sum_n gt        [C]
    precisions   = (intersection + EPS) / (pred_sum + EPS)
    recalls      = (intersection + EPS) / (gt_sum + EPS)

Sharding: rows split across 8 NeuronCores. Values are 0/1 indicators, so
the host re-encodes them losslessly as fp8_e4m3 (1 byte/elem) — 4x less
HBM traffic than the f32 originals (16.8 MiB/core, ~42 us at the ~410 GB/s
per-core DMA rate, vs 182 us for f32).

Host staging per core: x[tile=16, p=128, free=8224] fp8, where free is 32
groups of 257 cols: [pred(128) | ones(1) | gt(128)]. A group's 128 cols
are (class c, subrow r) pairs, col = c*8+r; its 128 partitions each hold
a distinct row, so one group covers 1024 rows.

Device pipeline per core:
  - Input DMAs ride the two HWDGE queues (sync + scalar engines) — the
    gpsimd SWDGE path costs ~9 us of descriptor-generation ramp-up.
    Last tile lands in 4 quarters so compute can chase the stream's tail.
    All 16 SBUF slots are resident — no recycling.
  - TensorE, per group: matmul lhsT = pred cols (128-wide), rhs =
    [ones | gt] (N=129), accumulating into ps_gram[128, 129]:
      col 0   = per-(c,r) pred sums   (weights x ones column)
      diag of cols 1..129 = per-(c,r) intersections
  - gt sums are split between the two engines that have slack: VectorE
    strided-reduces groups 0..15 of each tile (~2.2 us/tile); TensorE
    sums groups 16..31 with one ones-weight N=512 matmul per 4-group
    span into ps_sum2[1, 512] (~0.9 us/tile). Either engine alone would
    be the bottleneck (DVE: 4.4 us/tile measured; PE: +1.7 us/tile).
  - Epilogue: DVE masks ps_gram's diagonal (identity mask), copies col 0,
    folds ps_sum2; three tiny fp32 matmuls fold the partition axis; DVE
    copies psums to SBUF; DMA writes a [1, 400] partial vector.
  - Host: sums 8 cores' partials (exact integers in f64), folds subrows,
    applies the epsilon math.
"""

from contextlib import ExitStack

import numpy as np

N_CORES = 8
N_ROWS, C = 4194304, 16
ROWS_PER_CORE = N_ROWS // N_CORES  # 524288
EPS = np.float32(1e-6)

P = 128              # partitions; also pred/gt cols per group (16 classes x 8 subrows)
R_SUB = 8            # subrows folded into a group's column block
GCOLS = 2 * P + 1    # 257: [pred(128) | ones(1) | gt(128)]
GROUPS_PER_TILE = 32
N_TILES = ROWS_PER_CORE // (P * R_SUB * GROUPS_PER_TILE)  # 16
TILE_FREE = GROUPS_PER_TILE * GCOLS  # 8224
N_QUARTERS = 4       # last tile split so PE/DVE finish right after the stream
SPAN = 4             # groups per TensorE sum-matmul (4 x 128 = 512 = psum bank)
OUT_COLS = 2 * P + C + P  # 400: [I (c,r) | pred (c,r) | gt_dve (c) | gt_pe (c,r)]

_F8_ONE = np.uint8(0x38)  # 1.0 in float8_e4m3

_CACHE = {}
LAST_RUN = None  # BassKernelResults of the most recent run (for test harness)


def _build_nc(n_tiles=N_TILES, groups_per_tile=GROUPS_PER_TILE):
    import concourse.bass as bass
    import concourse.mybir as mybir

    f32 = mybir.dt.float32
    fp8 = mybir.dt.float8e4

    tile_free = groups_per_tile * GCOLS
    g_half = groups_per_tile // 2          # DVE reduces groups [0, g_half)
    spans = g_half // SPAN                 # TensorE sum-MM spans per tile
    gq = groups_per_tile // N_QUARTERS     # groups per quarter (last tile)
    n_segs = n_tiles + 1                   # gtacc segments (last tile -> 2)
    n_groups = n_tiles * groups_per_tile
    n_main_mms = n_groups + n_tiles * spans

    nc = bass.Bass()
    x_d = nc.dram_tensor("x", [n_tiles, P, tile_free], fp8, kind="ExternalInput")
    out_d = nc.dram_tensor("out", [1, OUT_COLS], f32, kind="ExternalOutput")

    ctx = ExitStack()
    with ctx:
        ones_f = ctx.enter_context(nc.sbuf_tensor("ones_f", [P, 1], f32))
        onesI = ctx.enter_context(nc.sbuf_tensor("onesI", [P, P], f32))
        ident = ctx.enter_context(nc.sbuf_tensor("ident", [P, P], f32))
        diagm = ctx.enter_context(nc.sbuf_tensor("diagm", [P, P + 1], f32))
        gtacc = ctx.enter_context(nc.sbuf_tensor("gtacc", [P, n_segs * C], f32))
        gt16 = ctx.enter_context(nc.sbuf_tensor("gt16", [P, C], f32))
        res = ctx.enter_context(nc.sbuf_tensor("res", [1, OUT_COLS], f32))
        slots = [
            ctx.enter_context(nc.sbuf_tensor(f"xt{t}", [P, tile_free], fp8))
            for t in range(n_tiles)
        ]

        ps_gram = ctx.enter_context(nc.psum_tensor([P, P + 1], f32))
        ps_sum2 = ctx.enter_context(nc.psum_tensor([1, SPAN * P], f32))
        ps_epiA = ctx.enter_context(nc.psum_tensor([1, P], f32))
        ps_epiB = ctx.enter_context(nc.psum_tensor([1, P], f32))
        ps_epiC = ctx.enter_context(nc.psum_tensor([1, C], f32))

        tsems = [
            ctx.enter_context(nc.semaphore(name=f"t{t}"))
            for t in range(n_tiles - 1)
        ]
        qsems = [
            ctx.enter_context(nc.semaphore(name=f"q{k}"))
            for k in range(N_QUARTERS)
        ]
        pe_sem = ctx.enter_context(nc.semaphore(name="pe"))
        dve_sem = ctx.enter_context(nc.semaphore(name="dve"))
        pool_sem = ctx.enter_context(nc.semaphore(name="pool"))
        v_sem = ctx.enter_context(nc.semaphore(name="vself"))
        out_sem = ctx.enter_context(nc.semaphore(name="outd"))
        block = ctx.enter_context(nc.Block())

        def grouped(slot):
            return slot[:, :].rearrange("p (f col) -> p f col",
                                        f=groups_per_tile)

        def gt_reduce_view(slot, f0, f1):
            # [p, c, f, r] view of the gt sections of groups [f0, f1)
            v = grouped(slot)[:, f0:f1, P + 1:GCOLS]
            return v.rearrange("p f (c r) -> p c f r", r=R_SUB)

        last = n_tiles - 1
        qf = tile_free // N_QUARTERS

        @block.sync
        def _(sync):
            for t in range(0, n_tiles - 1, 2):
                sync.dma_start(slots[t][:], x_d[t]).then_inc(tsems[t], 16)

        @block.scalar
        def _(scalar):
            for t in range(1, n_tiles - 1, 2):
                scalar.dma_start(slots[t][:], x_d[t]).then_inc(tsems[t], 16)
            for k in range(N_QUARTERS):
                scalar.dma_start(
                    slots[last][:, k * qf:(k + 1) * qf],
                    x_d[last][:, k * qf:(k + 1) * qf],
                ).then_inc(qsems[k], 16)

        @block.gpsimd
        def _(gpsimd):
            gpsimd.memset(onesI[:], 1.0).then_inc(pool_sem, 1)
            gpsimd.wait_ge(pool_sem, 1)  # Q7 cores race back-to-back ops
            gpsimd.affine_select(ident[:], onesI[:], [[1, P]],
                                 mybir.AluOpType.is_equal, 0.0,
                                 base=0,
                                 channel_multiplier=-1).then_inc(pool_sem, 1)
            # final output DMA after DVE lands everything in res
            gpsimd.wait_ge(dve_sem, 7)
            gpsimd.dma_start(out_d[:, :], res[:]).then_inc(out_sem, 16)
            gpsimd.wait_ge(out_sem, 16)

        @block.vector
        def _(vector):
            vector.memset(ones_f[:], 1.0)
            for t in range(n_tiles - 1):
                vector.wait_ge(tsems[t], 16)
                vector.tensor_reduce(
                    gtacc[:, t * C:(t + 1) * C],
                    gt_reduce_view(slots[t], 0, g_half),
                    axis=mybir.AxisListType.XY,
                    op=mybir.AluOpType.add).then_inc(v_sem, 1)
            # last tile: its DVE half arrives as quarters 0 and 1
            for k in range(2):
                vector.wait_ge(qsems[k], 16)
                seg = n_tiles - 1 + k
                vector.tensor_reduce(
                    gtacc[:, seg * C:(seg + 1) * C],
                    gt_reduce_view(slots[last], k * gq, (k + 1) * gq),
                    axis=mybir.AxisListType.XY,
                    op=mybir.AluOpType.add).then_inc(v_sem, 1)
            # self-sync: gtacc writes must land before the fold reads them
            vector.wait_ge(v_sem, n_segs)
            vector.tensor_reduce(
                gt16[:, :],
                gtacc[:, :].rearrange("p (s c) -> p c s", c=C),
                axis=mybir.AxisListType.X,
                op=mybir.AluOpType.add).then_inc(dve_sem, 1)
            # fold ps_sum2's span axis; mask ps_gram's diagonal; copy col 0
            vector.wait_ge(pool_sem, 2)
            vector.wait_ge(pe_sem, 1)
            vector.tensor_reduce(
                res[:, 2 * P + C:OUT_COLS],
                ps_sum2[:, :].rearrange("p (j m) -> p m j", m=P),
                axis=mybir.AxisListType.X,
                op=mybir.AluOpType.add).then_inc(dve_sem, 1)
            vector.tensor_mul(diagm[:, 1:P + 1], ps_gram[:, 1:P + 1],
                              ident[:]).then_inc(dve_sem, 1)
            vector.tensor_scalar_mul(diagm[:, 0:1], ps_gram[:, 0:1],
                                     1.0).then_inc(dve_sem, 1)
            # copy epilogue psums -> res after PE's fold matmuls
            vector.wait_ge(pe_sem, 2)
            vector.tensor_scalar_mul(res[:, 0:P], ps_epiA[:, :],
                                     1.0).then_inc(dve_sem, 1)
            vector.tensor_scalar_mul(res[:, P:2 * P], ps_epiB[:, :],
                                     1.0).then_inc(dve_sem, 1)
            vector.tensor_scalar_mul(res[:, 2 * P:2 * P + C], ps_epiC[:, :],
                                     1.0).then_inc(dve_sem, 1)

        @block.tensor
        def _(tensor):
            mm = [0, 0]  # gram count, sum count

            def gram(t, g):
                base = g * GCOLS
                inst = nc.tensor.matmul(
                    ps_gram[:, :],
                    slots[t][:, base:base + P],
                    slots[t][:, base + P:base + GCOLS],
                    start=(mm[0] == 0), stop=(mm[0] == n_groups - 1))
                mm[0] += 1
                return inst

            def gtsum(t, j):
                f0 = g_half + j * SPAN
                inst = nc.tensor.matmul(
                    ps_sum2[:, :],
                    slots[t][:, P:P + 1],  # group 0's staged ones column
                    grouped(slots[t])[:, f0:f0 + SPAN, P + 1:GCOLS],
                    start=(mm[1] == 0), stop=(mm[1] == n_tiles * spans - 1))
                mm[1] += 1
                return inst

            for t in range(n_tiles - 1):
                tensor.wait_ge(tsems[t], 16)
                for g in range(groups_per_tile):
                    gram(t, g)
                for j in range(spans):
                    gtsum(t, j)
            # last tile: chase the quarter DMAs; sum spans are
            # quarter-aligned (issue each span after its last quarter)
            for k in range(N_QUARTERS):
                tensor.wait_ge(qsems[k], 16)
                for g in range(k * gq, (k + 1) * gq):
                    gram(last, g)
                for j in range(spans):
                    span_last_g = g_half + j * SPAN + SPAN - 1
                    if k * gq <= span_last_g < (k + 1) * gq:
                        inst = gtsum(last, j)
            inst.then_inc(pe_sem, 1)
            assert mm[0] == n_groups and mm[1] == n_tiles * spans
            # epilogue: fold the partition axis of the three partial sets
            tensor.wait_ge(pool_sem, 2)
            tensor.wait_ge(dve_sem, 4)
            nc.tensor.matmul(ps_epiA[:, :], ones_f[:], diagm[:, 1:P + 1],
                             start=True, stop=True)
            nc.tensor.matmul(ps_epiB[:, :], diagm[:, 0:1], ident[:],
                             start=True, stop=True)
            nc.tensor.matmul(ps_epiC[:, :], ones_f[:], gt16[:, :],
                             start=True, stop=True).then_inc(pe_sem, 1)

    return nc


def _pack_core(pred_c, gt_c, n_tiles=N_TILES, groups_per_tile=GROUPS_PER_TILE):
    """Stage one core's rows as [n_tiles, P, tile_free] fp8 bytes (uint8)."""
    shp = (n_tiles, P, groups_per_tile, R_SUB, C)
    pc = np.asarray(pred_c).reshape(shp)
    gc = np.asarray(gt_c).reshape(shp)
    X = np.empty((n_tiles, P, groups_per_tile, GCOLS), np.uint8)
    # cols are (c, r) pairs, col = c*R_SUB + r -> transpose r and c
    X[..., 0:P] = (pc.transpose(0, 1, 2, 4, 3) != 0).reshape(
        n_tiles, P, groups_per_tile, P) * _F8_ONE
    X[..., P] = _F8_ONE
    X[..., P + 1:GCOLS] = (gc.transpose(0, 1, 2, 4, 3) != 0).reshape(
        n_tiles, P, groups_per_tile, P) * _F8_ONE
    return X.reshape(n_tiles, P, groups_per_tile * GCOLS)


def _unpack_out(partials):
    """partials: [OUT_COLS] f64 summed over cores -> (I, pred_sum, gt_sum)."""
    inter = partials[0:P].reshape(C, R_SUB).sum(axis=1)
    pred_sum = partials[P:2 * P].reshape(C, R_SUB).sum(axis=1)
    gt_sum = (partials[2 * P:2 * P + C]
              + partials[2 * P + C:OUT_COLS].reshape(C, R_SUB).sum(axis=1))
    return inter, pred_sum, gt_sum


def _get_nc():
    if "nc" not in _CACHE:
        _CACHE["nc"] = _build_nc()
    return _CACHE["nc"]


def kernel(pred, gt, **run_kwargs):
    global LAST_RUN
    import ml_dtypes
    from concourse.bass_utils import run_bass_kernel_spmd

    pred = np.asarray(pred)
    gt = np.asarray(gt)
    assert pred.shape == (N_ROWS, C) and gt.shape == (N_ROWS, C)

    in_maps = []
    for i in range(N_CORES):
        sl = slice(i * ROWS_PER_CORE, (i + 1) * ROWS_PER_CORE)
        X = _pack_core(pred[sl], gt[sl])
        in_maps.append({"x": X.view(ml_dtypes.float8_e4m3)})

    nc = _get_nc()
    br = run_bass_kernel_spmd(nc, in_maps, core_ids=list(range(N_CORES)),
                              **run_kwargs)
    LAST_RUN = br

    partials = np.stack([r["out"].reshape(OUT_COLS) for r in br.results])
    totals = partials.astype(np.float64).sum(axis=0)  # exact integers
    inter, pred_sum, gt_sum = _unpack_out(totals)
    inter = inter.astype(np.float32)
    pred_sum = pred_sum.astype(np.float32)
    gt_sum = gt_sum.astype(np.float32)

    recalls = (inter + EPS) / (gt_sum + EPS)
    precisions = (inter + EPS) / (pred_sum + EPS)
    return (precisions, recalls, inter, gt_sum, pred_sum)


# revision 15
# speedup vs baseline: 2.6747x; 1.0038x over previous
"""Trainium2 Bass kernel: per-class precision/recall sums via fp8 gram matmuls.

Computes, for pred/gt 0-1 indicator tensors of shape [N, C]:
    intersection = sum_n pred*gt   [C]
    pred_sum     = sum_n pred      [C]
    gt_sum       = sum_n gt        [C]
    precisions   = (intersection + EPS) / (pred_sum + EPS)
    recalls      = (intersection + EPS) / (gt_sum + EPS)

Sharding: rows split across 8 NeuronCores. Values are 0/1 indicators, so
the host re-encodes them losslessly as fp8_e4m3 (1 byte/elem) — 4x less
HBM traffic than the f32 originals (16.8 MiB/core, ~40 us at the ~420 GB/s
per-core DMA rate, vs 182 us for f32).

Host staging per core: x[tile=16, p=128, free=8224] fp8, where free is 32
groups of 257 cols: [pred(128) | ones(1) | gt(128)]. A group's 128 cols
are (class c, subrow r) pairs, col = c*8+r; its 128 partitions each hold
a distinct row, so one group covers 1024 rows.

Device pipeline per core:
  - Input DMAs ride the two HWDGE queues (sync + scalar engines) — the
    gpsimd SWDGE path costs ~9 us of descriptor-generation ramp-up.
    Last tile lands in 4 quarters so compute can chase the stream's tail.
    All 16 SBUF slots are resident — no recycling.
  - TensorE, per group: matmul lhsT = pred cols (128-wide), rhs =
    [ones | gt] (N=129), accumulating into ps_gram[128, 129]:
      col 0   = per-(c,r) pred sums   (weights x ones column)
      diag of cols 1..129 = per-(c,r) intersections
  - gt sums are split between the two engines that have slack: VectorE
    strided-reduces groups 0..15 of each tile (~2.3 us/tile) into gtacc;
    TensorE sums groups 16..31 with ones-weight N=512 matmuls (one per
    4-group span) into ps_sum2[1, 512]. The per-tile sum matmuls run
    back-to-back so the ones weights load once per tile and the next
    gram's 128-col weight load hides under their streaming.
  - No device epilogue: DVE copies ps_gram/ps_sum2 to SBUF (DMA cannot
    read PSUM) and the partial tensors go to HBM raw — o1[128, 129],
    o2[1, 512], o3 = gtacc[128, segs*16]. The host extracts the diag,
    folds the span/subrow/partition axes, sums the 8 cores' partials
    (exact integers in f64), and applies the epsilon math.
"""

from contextlib import ExitStack

import numpy as np

N_CORES = 8
N_ROWS, C = 4194304, 16
ROWS_PER_CORE = N_ROWS // N_CORES  # 524288
EPS = np.float32(1e-6)

P = 128              # partitions; also pred/gt cols per group (16 classes x 8 subrows)
R_SUB = 8            # subrows folded into a group's column block
GCOLS = 2 * P + 1    # 257: [pred(128) | ones(1) | gt(128)]
GROUPS_PER_TILE = 32
N_TILES = ROWS_PER_CORE // (P * R_SUB * GROUPS_PER_TILE)  # 16
TILE_FREE = GROUPS_PER_TILE * GCOLS  # 8224
N_QUARTERS = 4       # last tile split so PE/DVE finish right after the stream
SPAN = 4             # groups per TensorE sum-matmul (4 x 128 = 512 = psum bank)

_F8_ONE = np.uint8(0x38)  # 1.0 in float8_e4m3

_CACHE = {}
LAST_RUN = None  # BassKernelResults of the most recent run (for test harness)


def _build_nc(n_tiles=N_TILES, groups_per_tile=GROUPS_PER_TILE):
    import concourse.bass as bass
    import concourse.mybir as mybir

    f32 = mybir.dt.float32
    fp8 = mybir.dt.float8e4

    tile_free = groups_per_tile * GCOLS
    g_half = groups_per_tile // 2          # DVE reduces groups [0, g_half)
    spans = g_half // SPAN                 # TensorE sum-MM spans per tile
    gq = groups_per_tile // N_QUARTERS     # groups per quarter (last tile)
    n_segs = n_tiles + 1                   # gtacc segments (last tile -> 2)
    n_groups = n_tiles * groups_per_tile

    nc = bass.Bass()
    x_d = nc.dram_tensor("x", [n_tiles, P, tile_free], fp8, kind="ExternalInput")
    o1_d = nc.dram_tensor("o1", [P, P + 1], f32, kind="ExternalOutput")
    o2_d = nc.dram_tensor("o2", [1, SPAN * P], f32, kind="ExternalOutput")
    o3_d = nc.dram_tensor("o3", [P, n_segs * C], f32, kind="ExternalOutput")

    ctx = ExitStack()
    with ctx:
        gtacc = ctx.enter_context(nc.sbuf_tensor("gtacc", [P, n_segs * C], f32))
        gbuf = ctx.enter_context(nc.sbuf_tensor("gbuf", [P, P + 1], f32))
        s2buf = ctx.enter_context(nc.sbuf_tensor("s2buf", [1, SPAN * P], f32))
        slots = [
            ctx.enter_context(nc.sbuf_tensor(f"xt{t}", [P, tile_free], fp8))
            for t in range(n_tiles)
        ]

        ps_gram = ctx.enter_context(nc.psum_tensor([P, P + 1], f32))
        ps_sum2 = ctx.enter_context(nc.psum_tensor([1, SPAN * P], f32))

        tsems = [
            ctx.enter_context(nc.semaphore(name=f"t{t}"))
            for t in range(n_tiles - 1)
        ]
        qsems = [
            ctx.enter_context(nc.semaphore(name=f"q{k}"))
            for k in range(N_QUARTERS)
        ]
        pe_sem = ctx.enter_context(nc.semaphore(name="pe"))
        v_sem = ctx.enter_context(nc.semaphore(name="vself"))
        out_sem = ctx.enter_context(nc.semaphore(name="outd"))
        block = ctx.enter_context(nc.Block())

        def grouped(slot):
            return slot[:, :].rearrange("p (f col) -> p f col",
                                        f=groups_per_tile)

        def gt_reduce_view(slot, f0, f1):
            # [p, c, f, r] view of the gt sections of groups [f0, f1)
            v = grouped(slot)[:, f0:f1, P + 1:GCOLS]
            return v.rearrange("p f (c r) -> p c f r", r=R_SUB)

        last = n_tiles - 1
        qf = tile_free // N_QUARTERS

        @block.sync
        def _(sync):
            for t in range(0, n_tiles - 1, 2):
                sync.dma_start(slots[t][:], x_d[t]).then_inc(tsems[t], 16)
            # partial outputs, once DVE finished its reduces + psum copies
            sync.wait_ge(v_sem, n_segs + 2)
            sync.dma_start(o1_d[:, :], gbuf[:]).then_inc(out_sem, 16)
            sync.dma_start(o2_d[:, :], s2buf[:]).then_inc(out_sem, 16)
            sync.dma_start(o3_d[:, :], gtacc[:]).then_inc(out_sem, 16)
            sync.wait_ge(out_sem, 48)

        @block.scalar
        def _(scalar):
            for t in range(1, n_tiles - 1, 2):
                scalar.dma_start(slots[t][:], x_d[t]).then_inc(tsems[t], 16)
            for k in range(N_QUARTERS):
                scalar.dma_start(
                    slots[last][:, k * qf:(k + 1) * qf],
                    x_d[last][:, k * qf:(k + 1) * qf],
                ).then_inc(qsems[k], 16)

        @block.vector
        def _(vector):
            for t in range(n_tiles - 1):
                vector.wait_ge(tsems[t], 16)
                vector.tensor_reduce(
                    gtacc[:, t * C:(t + 1) * C],
                    gt_reduce_view(slots[t], 0, g_half),
                    axis=mybir.AxisListType.XY,
                    op=mybir.AluOpType.add).then_inc(v_sem, 1)
            # last tile: its DVE half arrives as quarters 0 and 1
            for k in range(2):
                vector.wait_ge(qsems[k], 16)
                seg = n_tiles - 1 + k
                vector.tensor_reduce(
                    gtacc[:, seg * C:(seg + 1) * C],
                    gt_reduce_view(slots[last], k * gq, (k + 1) * gq),
                    axis=mybir.AxisListType.XY,
                    op=mybir.AluOpType.add).then_inc(v_sem, 1)
            # copy the psum partials to SBUF so DMA can ship them
            vector.wait_ge(pe_sem, 1)
            vector.tensor_scalar_mul(gbuf[:, :], ps_gram[:, :],
                                     1.0).then_inc(v_sem, 1)
            vector.tensor_scalar_mul(s2buf[:, :], ps_sum2[:, :],
                                     1.0).then_inc(v_sem, 1)

        @block.tensor
        def _(tensor):
            mm = [0, 0]  # gram count, sum count

            def gram(t, g):
                base = g * GCOLS
                inst = nc.tensor.matmul(
                    ps_gram[:, :],
                    slots[t][:, base:base + P],
                    slots[t][:, base + P:base + GCOLS],
                    start=(mm[0] == 0), stop=(mm[0] == n_groups - 1))
                mm[0] += 1
                return inst

            def gtsum(t, j):
                f0 = g_half + j * SPAN
                inst = nc.tensor.matmul(
                    ps_sum2[:, :],
                    slots[t][:, P:P + 1],  # group 0's staged ones column
                    grouped(slots[t])[:, f0:f0 + SPAN, P + 1:GCOLS],
                    start=(mm[1] == 0), stop=(mm[1] == n_tiles * spans - 1))
                mm[1] += 1
                return inst

            for t in range(n_tiles - 1):
                tensor.wait_ge(tsems[t], 16)
                # sum matmuls back-to-back: one ones-LDW per tile, and the
                # first gram's 128-col LDW pulls ahead under their streaming
                for j in range(spans):
                    gtsum(t, j)
                for g in range(groups_per_tile):
                    gram(t, g)
            # last tile: chase the quarter DMAs; sum spans are
            # quarter-aligned (issue each span after its last quarter)
            for k in range(N_QUARTERS):
                tensor.wait_ge(qsems[k], 16)
                for j in range(spans):
                    span_last_g = g_half + j * SPAN + SPAN - 1
                    if k * gq <= span_last_g < (k + 1) * gq:
                        gtsum(last, j)
                for g in range(k * gq, (k + 1) * gq):
                    final = gram(last, g)
            # the final main-loop instruction carries the completion inc
            final.then_inc(pe_sem, 1)
            assert mm[0] == n_groups and mm[1] == n_tiles * spans

    return nc


def _pack_core(pred_c, gt_c, n_tiles=N_TILES, groups_per_tile=GROUPS_PER_TILE):
    """Stage one core's rows as [n_tiles, P, tile_free] fp8 bytes (uint8)."""
    shp = (n_tiles, P, groups_per_tile, R_SUB, C)
    pc = np.asarray(pred_c).reshape(shp)
    gc = np.asarray(gt_c).reshape(shp)
    X = np.empty((n_tiles, P, groups_per_tile, GCOLS), np.uint8)
    # cols are (c, r) pairs, col = c*R_SUB + r -> transpose r and c
    X[..., 0:P] = (pc.transpose(0, 1, 2, 4, 3) != 0).reshape(
        n_tiles, P, groups_per_tile, P) * _F8_ONE
    X[..., P] = _F8_ONE
    X[..., P + 1:GCOLS] = (gc.transpose(0, 1, 2, 4, 3) != 0).reshape(
        n_tiles, P, groups_per_tile, P) * _F8_ONE
    return X.reshape(n_tiles, P, groups_per_tile * GCOLS)


def _unpack_out(o1, o2, o3):
    """Fold one core's raw partials (f64) -> (I, pred_sum, gt_sum), each [C]."""
    diag = o1[np.arange(P), 1 + np.arange(P)]        # I by (c, r)
    inter = diag.reshape(C, R_SUB).sum(axis=1)
    pred_sum = o1[:, 0].reshape(C, R_SUB).sum(axis=1)
    gt_pe = o2.reshape(SPAN, P).sum(axis=0).reshape(C, R_SUB).sum(axis=1)
    gt_dve = o3.reshape(P, -1, C).sum(axis=(0, 1))
    return inter, pred_sum, gt_dve + gt_pe


def _get_nc():
    if "nc" not in _CACHE:
        _CACHE["nc"] = _build_nc()
    return _CACHE["nc"]


def kernel(pred, gt, **run_kwargs):
    global LAST_RUN
    import ml_dtypes
    from concourse.bass_utils import run_bass_kernel_spmd

    pred = np.asarray(pred)
    gt = np.asarray(gt)
    assert pred.shape == (N_ROWS, C) and gt.shape == (N_ROWS, C)

    in_maps = []
    for i in range(N_CORES):
        sl = slice(i * ROWS_PER_CORE, (i + 1) * ROWS_PER_CORE)
        X = _pack_core(pred[sl], gt[sl])
        in_maps.append({"x": X.view(ml_dtypes.float8_e4m3)})

    nc = _get_nc()
    br = run_bass_kernel_spmd(nc, in_maps, core_ids=list(range(N_CORES)),
                              **run_kwargs)
    LAST_RUN = br

    inter = np.zeros(C)
    pred_sum = np.zeros(C)
    gt_sum = np.zeros(C)
    for r in br.results:
        i_, p_, g_ = _unpack_out(r["o1"].astype(np.float64),
                                 r["o2"].astype(np.float64).reshape(-1),
                                 r["o3"].astype(np.float64))
        inter += i_
        pred_sum += p_
        gt_sum += g_
    inter = inter.astype(np.float32)
    pred_sum = pred_sum.astype(np.float32)
    gt_sum = gt_sum.astype(np.float32)

    recalls = (inter + EPS) / (gt_sum + EPS)
    precisions = (inter + EPS) / (pred_sum + EPS)
    return (precisions, recalls, inter, gt_sum, pred_sum)
